# revision 25
# baseline (speedup 1.0000x reference)
"""Trainium2 Bass kernel for nn_ACEEmbedAVD (gnn_message_passing).

Strategy:
  Host: nodes are packed into 128-partition x 8-tile "blocks" (1024 edge
  slots). Each node owns ceil(degree/4) capacity-4 slot units; a unit is
  (partition p, half-group g) holding that node's edges in tiles g*4..g*4+3.
  The scatter matrix M[p, local_node] is therefore constant across the 4
  tiles of a half-group -> built once per half-group (8x less work than a
  per-tile onehot). Unused slots carry r=0 dummy edges whose contribution
  is exactly zero (env factor). No collectives: blocks are sharded across
  the 8 cores; host gathers output rows by node.

  Device (per core), t-innermost layouts for DVE 2x modes:
    pass A (sqrt ACT table): x2=|r|^2; s=sqrt(x2/8); env=relu(1-x2/8) bf16;
      v = r/sqrt(x2+64/289) bf16
    pass B (trig ACT table): rad_c = cos(pi*c*s)*env via round-to-nearest
      big-constant range reduction + Abs + Sin; phi (10x8 feat, bf16);
      M = is_equal(owner, iota32) per half-group; per block
      A_blk (32n, 80f) += M^T @ phi_t in PSUM, col-tiled so a quad's 4
      blocks land in one (128,80) PSUM tile; PE-transpose -> (80,128);
      stage2: B = A^T W_blockdiag (80->512) -> bf16 out rows.

  Host post: rows -> B_a (128), B_v (3x64 -> N,64,3),
  B_d (6 sym pairs x32 -> N,32,3,3 mirrored).
"""

import sys

if "/opt/trn_rl_repo" not in sys.path:
    sys.path.insert(0, "/opt/trn_rl_repo")

import numpy as np
import ml_dtypes

N_NODES = 50000
N_CORES = 8
BLK = 32            # output nodes per block (M columns)
CAP = 4             # edges per slot unit (tiles per half-group)
TPB = 8             # tiles per block
GPB = 2             # half-groups per block
UPB = 128 * GPB     # slot units per block
NF = 80             # phi features
OUTW = 512
CHUNK = 8           # quads per elementwise batch
T = 32              # tiles per quad (4 blocks x 8 tiles)

RND_C = 12582912.0   # 1.5 * 2**23: (x + C) - C == round-to-nearest(x)

_BF = ml_dtypes.bfloat16

_compiled_cache = {}


def _chunk_list(nquads):
    out = []
    q = 0
    while q < nquads:
        ch = min(CHUNK, nquads - q)
        out.append((q, ch))
        q += ch
    return out


def _build(nquads):
    from concourse import bacc, tile, mybir
    from concourse.tile_rust import add_dep_helper

    AF = mybir.ActivationFunctionType
    OP = mybir.AluOpType
    F32 = mybir.dt.float32
    BF16 = mybir.dt.bfloat16
    F16 = mybir.dt.float16

    TC = CHUNK * T        # tiles per full chunk
    GPC = CHUNK * 8       # half-groups per full chunk
    NGRP = nquads * 8     # half-groups per core

    nc = bacc.Bacc("TRN2", target_bir_lowering=False, debug=False)

    def rc(value, dtype=F32):
        key = (dtype, value)
        if key not in nc.const_aps.aps:
            t = nc.alloc_sbuf_tensor(f"c-{dtype.name}-{value}", [128, 1], dtype)
            nc.gpsimd.memset(t.ap(), value)
            nc.const_aps.aps[key] = t.ap()

    rc(64.0 / 289.0)
    rc(float(np.pi / 2))

    r_in = nc.dram_tensor("r", [128, 3 * nquads * T], F32, kind="ExternalInput")
    ow_in = nc.dram_tensor("ow", [128, NGRP], BF16, kind="ExternalInput")
    cv_in = nc.dram_tensor("cv", [1, 7, TC], F16, kind="ExternalInput")
    io_in = nc.dram_tensor("io", [1, BLK, GPC], BF16, kind="ExternalInput")
    id_in = nc.dram_tensor("idm", [128, 128], BF16, kind="ExternalInput")
    w_in = nc.dram_tensor("w", [NF, OUTW], BF16, kind="ExternalInput")
    out_p = nc.dram_tensor("out", [nquads * 128, OUTW], BF16, kind="ExternalOutput")

    chunks = _chunk_list(nquads)

    with tile.TileContext(nc) as tc:
        with (
            tc.tile_pool(name="const", bufs=1) as cpool,
            tc.tile_pool(name="storeV", bufs=len(chunks)) as poolV,
            tc.tile_pool(name="storeS", bufs=len(chunks)) as poolS,
            tc.tile_pool(name="storeE", bufs=len(chunks)) as poolE,
            tc.tile_pool(name="workA", bufs=3) as wa,
            tc.tile_pool(name="workB", bufs=2) as wb,
            tc.tile_pool(name="workC", bufs=3) as wc,
            tc.tile_pool(name="outB", bufs=3) as ob,
            tc.tile_pool(name="psum1", bufs=3, space="PSUM") as ps1,
            tc.tile_pool(name="psumT", bufs=2, space="PSUM") as psT,
            tc.tile_pool(name="psum2", bufs=2, space="PSUM") as ps2,
        ):
            cvt = cpool.tile([128, 7, TC], F16)
            nc.scalar.dma_start(out=cvt[:], in_=cv_in[:].to_broadcast([128, 7, TC]))
            iot = cpool.tile([128, BLK, GPC], BF16)
            nc.scalar.dma_start(out=iot[:], in_=io_in[:].to_broadcast([128, BLK, GPC]))
            idm = cpool.tile([128, 128], BF16)
            nc.scalar.dma_start(out=idm[:], in_=id_in[:])
            wsb = cpool.tile([NF, OUTW], BF16)
            nc.scalar.dma_start(out=wsb[:], in_=w_in[:])

            # ---------------- pass A: sqrt-family ----------------
            stores = []
            last_passA_act = None
            for q0, ch in chunks:
                tcs = ch * T
                rt = wa.tile([128, 3, 1, tcs], F32, tag="rt")
                for qq in range(ch):
                    nc.sync.dma_start(
                        out=rt[:, :, 0, qq * T : (qq + 1) * T],
                        in_=r_in[:, 3 * (q0 + qq) * T : 3 * (q0 + qq + 1) * T].rearrange(
                            "p (c t) -> p c t", c=3
                        ),
                    )
                sq = wa.tile([128, 3, tcs], F32, tag="sq")
                nc.scalar.activation(sq[:], rt[:, :, 0, :], AF.Square)
                x2 = wa.tile([128, 1, tcs], F32, tag="x2")
                nc.vector.tensor_tensor(x2[:], sq[:, 0:1, :], sq[:, 1:2, :], OP.add)
                nc.vector.tensor_tensor(x2[:], x2[:], sq[:, 2:3, :], OP.add)

                st = poolS.tile([128, 1, tcs], F16, tag="s")
                nc.scalar.activation(st[:], x2[:], AF.Sqrt, scale=0.125)
                env = poolE.tile([128, 1, 1, tcs], BF16, tag="env")
                nc.scalar.activation(
                    env[:, :, 0, :], x2[:], AF.Relu, scale=-0.125, bias=1.0
                )
                u = wa.tile([128, 1, tcs], F32, tag="u")
                ua = nc.scalar.activation(u[:], x2[:], AF.Sqrt, bias=64.0 / 289.0)
                last_passA_act = ua
                qr = wa.tile([128, 1, 1, tcs], F32, tag="qr")
                nc.vector.reciprocal(qr[:, :, 0, :], u[:])

                vt = poolV.tile([128, 3, 1, tcs], BF16, tag="v")
                nc.vector.tensor_tensor(
                    vt[:], rt[:], qr[:].to_broadcast([128, 3, 1, tcs]), OP.mult
                )
                stores.append((vt, st, env))

            # ---------------- pass B: trig + matmuls ----------------
            for ci, (q0, ch) in enumerate(chunks):
                tcs = ch * T
                gpc = ch * 8
                vt, st, env = stores[ci]
                own = wc.tile([128, 1, gpc], BF16, tag="own")
                nc.sync.dma_start(
                    out=own[:, 0, :], in_=ow_in[:, q0 * 8 : (q0 + ch) * 8]
                )

                h = wc.tile([128, 7, tcs], F16, tag="h")
                nc.vector.tensor_tensor(
                    h[:], st[:].to_broadcast([128, 7, tcs]), cvt[:, :, :tcs], OP.mult
                )
                rnd = wc.tile([128, 7, tcs], F16, tag="rnd")
                nc.vector.tensor_scalar(
                    rnd[:], h[:], RND_C, RND_C, OP.add, OP.subtract
                )
                z = wc.tile([128, 7, tcs], F16, tag="z")
                nc.vector.tensor_tensor(z[:], h[:], rnd[:], OP.subtract)
                ab = wc.tile([128, 7, tcs], F16, tag="ab")
                ai = nc.scalar.activation(ab[:], z[:], AF.Abs)
                add_dep_helper(
                    ai.ins, last_passA_act.ins, sync=False,
                    reason="keep trig-set ACT ops after all sqrt-set ACT ops",
                )
                radp = wc.tile([128, 1, 7, tcs], BF16, tag="radp")
                nc.scalar.activation(
                    radp[:, 0, :, :],
                    ab[:],
                    AF.Sin,
                    scale=float(-2 * np.pi),
                    bias=float(np.pi / 2),
                )

                phi = wb.tile([128, 10, 8, tcs], BF16, tag="phi")
                nc.vector.tensor_copy(phi[:, 0:1, 0:1, :], env[:])
                nc.vector.tensor_tensor(
                    phi[:, 0:1, 1:8, :], radp[:],
                    env[:].to_broadcast([128, 1, 7, tcs]), OP.mult
                )
                nc.vector.tensor_tensor(
                    phi[:, 1:4],
                    vt[:].to_broadcast([128, 3, 8, tcs]),
                    phi[:, 0:1].to_broadcast([128, 3, 8, tcs]),
                    OP.mult,
                )
                nc.vector.tensor_tensor(
                    phi[:, 4:7],
                    vt[:, 0:1].to_broadcast([128, 3, 8, tcs]),
                    phi[:, 1:4],
                    OP.mult,
                )
                nc.vector.tensor_tensor(
                    phi[:, 7:9],
                    vt[:, 1:2].to_broadcast([128, 2, 8, tcs]),
                    phi[:, 2:4],
                    OP.mult,
                )
                nc.vector.tensor_tensor(
                    phi[:, 9:10],
                    vt[:, 2:3].to_broadcast([128, 1, 8, tcs]),
                    phi[:, 3:4],
                    OP.mult,
                )

                # scatter matrices, one per half-group
                M = wb.tile([128, BLK, gpc], BF16, tag="M")
                nc.vector.tensor_tensor(
                    M[:], own[:].to_broadcast([128, BLK, gpc]), iot[:, :, :gpc],
                    OP.is_equal,
                )

                for pq in range(0, ch, 2):
                    npair = min(2, ch - pq)
                    psq = ps1.tile([128, 2, NF], F32)
                    for par in range(npair):
                        qq = pq + par
                        for g in range(GPB):
                            for tt in range(CAP):
                                for qb in range(4):
                                    grp = qq * 8 + qb * GPB + g
                                    t = qq * T + qb * TPB + g * CAP + tt
                                    nc.tensor.matmul(
                                        psq[qb * BLK : (qb + 1) * BLK, par, :],
                                        M[:, :, grp],
                                        phi[:, :, :, t],
                                        start=(g == 0 and tt == 0),
                                        stop=(g == GPB - 1 and tt == CAP - 1),
                                        tile_position=(0, qb * BLK),
                                    )
                    aq = ob.tile([128, 2, NF], BF16, tag="aq")
                    nc.scalar.copy(aq[:, :npair, :], psq[:, :npair, :])

                    for par in range(npair):
                        qq = pq + par
                        pst = psT.tile([NF, 128], BF16)
                        nc.tensor.transpose(pst[:], aq[:, par, :], idm[:])
                        a2 = ob.tile([NF, 128], BF16, tag="a2")
                        nc.scalar.copy(a2[:], pst[:])

                        po = ps2.tile([128, OUTW], F32)
                        nc.tensor.matmul(po[:], a2[:], wsb[:], start=True, stop=True)
                        osb = ob.tile([128, OUTW], BF16, tag="osb")
                        nc.scalar.copy(osb[:], po[:])
                        nc.sync.dma_start(
                            out=out_p[(q0 + qq) * 128 : (q0 + qq + 1) * 128, :],
                            in_=osb[:],
                        )

    nc.compile()
    return nc


def _get_compiled(nquads):
    if nquads not in _compiled_cache:
        _compiled_cache[nquads] = _build(nquads)
    return _compiled_cache[nquads]


def _pack_nodes(counts, slack=1.015):
    """Pack nodes into blocks: node n takes U=ceil(d/4) capacity-4 units;
    each block holds <=32 nodes and <=256 units. Serpentine deal by U desc
    + fixup; retries with more blocks if packing fails."""
    U = (counts + CAP - 1) // CAP
    total_units = int(U.sum())
    n = counts.shape[0]
    nb_min = max(1, int(np.ceil(total_units / UPB)))
    nb = int(np.ceil(nb_min * slack / 32.0)) * 32
    while nb * BLK < n:
        nb += 32

    order = np.argsort(-U, kind="stable")
    blk_of = np.empty(n, np.int64)
    pos = 0
    ri = 0
    while pos < n:
        take = min(nb, n - pos)
        ids = order[pos : pos + take]
        blocks = np.arange(take) if ri % 2 == 0 else (nb - 1 - np.arange(take))
        blk_of[ids] = blocks
        pos += take
        ri += 1

    bu = np.bincount(blk_of, weights=U.astype(np.float64), minlength=nb).astype(np.int64)
    bn = np.bincount(blk_of, minlength=nb)

    if (bu > UPB).any():
        order_small = order[::-1]
        space = list(np.argsort(bu))
        for b in np.nonzero(bu > UPB)[0]:
            members = order_small[blk_of[order_small] == b]
            k = 0
            while bu[b] > UPB and k < len(members):
                node = members[k]
                k += 1
                un = int(U[node])
                for tb in space:
                    if tb != b and bu[tb] + un <= UPB and bn[tb] < BLK:
                        blk_of[node] = tb
                        bu[b] -= un
                        bu[tb] += un
                        bn[b] -= 1
                        bn[tb] += 1
                        break
                else:
                    return _pack_nodes(counts, slack=slack + 0.01)

    ord2 = np.lexsort((np.arange(n), blk_of))
    blk_sorted = blk_of[ord2]
    bnds = np.append(np.searchsorted(blk_sorted, np.arange(nb)), n)
    sizes = np.diff(bnds)
    Uo = U[ord2]
    cum = np.cumsum(Uo) - Uo
    base_unit = np.empty(n, np.int64)
    local_id = np.empty(n, np.int64)
    base_unit[ord2] = cum - np.repeat(cum[bnds[:-1]], sizes)
    local_id[ord2] = np.arange(n) - np.repeat(bnds[:-1], sizes)
    assert base_unit.max() < UPB and local_id.max() < BLK
    return nb, blk_of, base_unit, local_id


def _preprocess(r_ij, src):
    E = src.shape[0]
    src = np.asarray(src).astype(np.int64).ravel()
    r_ij = np.ascontiguousarray(np.asarray(r_ij, dtype=np.float32))

    counts = np.bincount(src, minlength=N_NODES)
    nb, blk_of, base_unit, local_id = _pack_nodes(counts)
    nbc = nb // N_CORES
    nquads = nbc // 4

    order = np.argsort(src, kind="stable")
    src_s = src[order]
    r_s = r_ij[order]
    starts = np.zeros(N_NODES + 1, np.int64)
    starts[1:] = np.cumsum(counts)
    rank = np.arange(E, dtype=np.int64) - starts[src_s]

    blk = blk_of[src_s]
    unit = base_unit[src_s] + rank // CAP
    g = unit // 128
    p = unit % 128
    tt = g * CAP + rank % CAP
    core = blk // nbc
    bl = blk - core * nbc
    quad = bl // 4
    gt = quad * T + (bl % 4) * TPB + tt

    # pad slots get r=(4,0,0): x2=16 -> env=relu(1-2)=0 -> phi==0 exactly
    r_tmp = np.zeros((N_CORES, 128, 3, nquads * T), np.float32)
    r_tmp[:, :, 0, :] = 4.0
    r_tmp[core, p, :, gt] = r_s
    r_dev = np.ascontiguousarray(
        r_tmp.reshape(N_CORES, 128, 3, nquads, T)
        .transpose(0, 1, 3, 2, 4)
        .reshape(N_CORES, 128, 3 * nquads * T)
    )

    # owner table per (partition, half-group)
    ow = np.full((N_CORES, 128, nbc * GPB), 99.0, np.float32)
    U = (counts + CAP - 1) // CAP
    nodes_rep = np.repeat(np.arange(N_NODES), U)
    un_off = np.arange(U.sum()) - np.repeat(np.cumsum(U) - U, U)
    un = base_unit[nodes_rep] + un_off
    ncore = blk_of // nbc
    nbl = blk_of - ncore * nbc
    ow[ncore[nodes_rep], un % 128, nbl[nodes_rep] * GPB + un // 128] = local_id[
        nodes_rep
    ]

    rows = ncore * (nquads * 128) + (nbl // 4) * 128 + (nbl % 4) * BLK + local_id
    return r_dev, ow.astype(_BF), nquads, rows


def _build_w(W_a, W_v, W_d):
    w = np.zeros((NF, OUTW), np.float32)
    w[0:8, 0:128] = W_a
    for t in range(3):
        w[(1 + t) * 8 : (2 + t) * 8, 128 + 64 * t : 128 + 64 * (t + 1)] = W_v
    for qi in range(6):
        w[(4 + qi) * 8 : (5 + qi) * 8, 320 + 32 * qi : 320 + 32 * (qi + 1)] = W_d
    return w.astype(_BF)


def _make_inputs(r_dev, ow_dev, nquads, W_a, W_v, W_d):
    TC = CHUNK * T
    GPC = CHUNK * 8
    cv = np.ascontiguousarray(
        np.broadcast_to((np.arange(1, 8, dtype=np.float16) * 0.5)[None, :, None], (1, 7, TC))
    )
    io = np.ascontiguousarray(
        np.broadcast_to(np.arange(BLK, dtype=np.float32)[None, :, None], (1, BLK, GPC))
    ).astype(_BF)
    idm = np.eye(128, dtype=np.float32).astype(_BF)
    w = _build_w(np.asarray(W_a, np.float32), np.asarray(W_v, np.float32),
                 np.asarray(W_d, np.float32))
    return [
        dict(r=r_dev[i], ow=ow_dev[i], cv=cv, io=io, idm=idm, w=w)
        for i in range(N_CORES)
    ]


def kernel(r_ij, src, W_a, W_v, W_d, n_nodes):
    from concourse.bass_utils import run_bass_kernel_spmd

    r_dev, ow_dev, nquads, rows = _preprocess(r_ij, src)
    nc = _get_compiled(nquads)
    in_maps = _make_inputs(r_dev, ow_dev, nquads, W_a, W_v, W_d)
    res = run_bass_kernel_spmd(nc, in_maps, core_ids=list(range(N_CORES)))
    full = np.concatenate(
        [np.asarray(res.results[i]["out"]) for i in range(N_CORES)], axis=0
    ).astype(np.float32)[rows]

    N = N_NODES
    B_a = np.ascontiguousarray(full[:, :128])
    B_v = np.ascontiguousarray(
        full[:, 128:320].reshape(N, 3, 64).transpose(0, 2, 1)
    )
    B_d6 = full[:, 320:512].reshape(N, 6, 32)
    pmap = np.array([[0, 1, 2], [1, 3, 4], [2, 4, 5]])
    B_d = np.ascontiguousarray(B_d6[:, pmap, :].transpose(0, 3, 1, 2))
    return B_a, B_v, B_d


# revision 26
# speedup vs baseline: 1.0191x; 1.0191x over previous
"""Trainium2 Bass kernel for nn_ACEEmbedAVD (gnn_message_passing).

Strategy:
  Host: nodes are packed into 128-partition x 8-tile "blocks" (1024 edge
  slots). Each node owns ceil(degree/4) capacity-4 slot units; a unit is
  (partition p, half-group g) holding that node's edges in tiles g*4..g*4+3.
  The scatter matrix M[p, local_node] is therefore constant across the 4
  tiles of a half-group -> built once per half-group (8x less work than a
  per-tile onehot). Unused slots carry r=0 dummy edges whose contribution
  is exactly zero (env factor). No collectives: blocks are sharded across
  the 8 cores; host gathers output rows by node.

  Device (per core), t-innermost layouts for DVE 2x modes:
    pass A (sqrt ACT table): x2=|r|^2; s=sqrt(x2/8); env=relu(1-x2/8) bf16;
      v = r/sqrt(x2+64/289) bf16
    pass B (trig ACT table): rad_c = cos(pi*c*s)*env via round-to-nearest
      big-constant range reduction + Abs + Sin; phi (10x8 feat, bf16);
      M = is_equal(owner, iota32) per half-group; per block
      A_blk (32n, 80f) += M^T @ phi_t in PSUM, col-tiled so a quad's 4
      blocks land in one (128,80) PSUM tile; PE-transpose -> (80,128);
      stage2: B = A^T W_blockdiag (80->512) -> bf16 out rows.

  Host post: rows -> B_a (128), B_v (3x64 -> N,64,3),
  B_d (6 sym pairs x32 -> N,32,3,3 mirrored).
"""

import sys

if "/opt/trn_rl_repo" not in sys.path:
    sys.path.insert(0, "/opt/trn_rl_repo")

import numpy as np
import ml_dtypes

N_NODES = 50000
N_CORES = 8
BLK = 32            # output nodes per block (M columns)
CAP = 4             # edges per slot unit (tiles per half-group)
TPB = 8             # tiles per block
GPB = 2             # half-groups per block
UPB = 128 * GPB     # slot units per block
NF = 80             # phi features
OUTW = 512
CHUNK = 6           # quads per elementwise batch
T = 32              # tiles per quad (4 blocks x 8 tiles)

RND_C = 12582912.0   # 1.5 * 2**23: (x + C) - C == round-to-nearest(x)

_BF = ml_dtypes.bfloat16

_compiled_cache = {}


def _chunk_list(nquads):
    out = []
    q = 0
    while q < nquads:
        ch = min(CHUNK, nquads - q)
        out.append((q, ch))
        q += ch
    return out


def _build(nquads):
    from concourse import bacc, tile, mybir
    from concourse.tile_rust import add_dep_helper

    AF = mybir.ActivationFunctionType
    OP = mybir.AluOpType
    F32 = mybir.dt.float32
    BF16 = mybir.dt.bfloat16
    F16 = mybir.dt.float16

    TC = CHUNK * T        # tiles per full chunk
    GPC = CHUNK * 8       # half-groups per full chunk
    NGRP = nquads * 8     # half-groups per core

    nc = bacc.Bacc("TRN2", target_bir_lowering=False, debug=False)

    def rc(value, dtype=F32):
        key = (dtype, value)
        if key not in nc.const_aps.aps:
            t = nc.alloc_sbuf_tensor(f"c-{dtype.name}-{value}", [128, 1], dtype)
            nc.gpsimd.memset(t.ap(), value)
            nc.const_aps.aps[key] = t.ap()

    rc(64.0 / 289.0)
    rc(float(np.pi / 2))

    r_in = nc.dram_tensor("r", [128, 3 * nquads * T], F32, kind="ExternalInput")
    ow_in = nc.dram_tensor("ow", [128, NGRP], BF16, kind="ExternalInput")
    cv_in = nc.dram_tensor("cv", [1, 7, TC], F16, kind="ExternalInput")
    io_in = nc.dram_tensor("io", [1, BLK, GPC], BF16, kind="ExternalInput")
    id_in = nc.dram_tensor("idm", [128, 128], BF16, kind="ExternalInput")
    w_in = nc.dram_tensor("w", [NF, OUTW], BF16, kind="ExternalInput")
    out_p = nc.dram_tensor("out", [nquads * 128, OUTW], BF16, kind="ExternalOutput")

    chunks = _chunk_list(nquads)

    with tile.TileContext(nc) as tc:
        with (
            tc.tile_pool(name="const", bufs=1) as cpool,
            tc.tile_pool(name="storeV", bufs=len(chunks)) as poolV,
            tc.tile_pool(name="storeS", bufs=len(chunks)) as poolS,
            tc.tile_pool(name="storeE", bufs=len(chunks)) as poolE,
            tc.tile_pool(name="workA", bufs=3) as wa,
            tc.tile_pool(name="workB", bufs=2) as wb,
            tc.tile_pool(name="workC", bufs=3) as wc,
            tc.tile_pool(name="outB", bufs=3) as ob,
            tc.tile_pool(name="psum1", bufs=3, space="PSUM") as ps1,
            tc.tile_pool(name="psumT", bufs=2, space="PSUM") as psT,
            tc.tile_pool(name="psum2", bufs=2, space="PSUM") as ps2,
        ):
            cvt = cpool.tile([128, 7, TC], F16)
            nc.scalar.dma_start(out=cvt[:], in_=cv_in[:].to_broadcast([128, 7, TC]))
            iot = cpool.tile([128, BLK, GPC], BF16)
            nc.scalar.dma_start(out=iot[:], in_=io_in[:].to_broadcast([128, BLK, GPC]))
            idm = cpool.tile([128, 128], BF16)
            nc.scalar.dma_start(out=idm[:], in_=id_in[:])
            wsb = cpool.tile([NF, OUTW], BF16)
            nc.scalar.dma_start(out=wsb[:], in_=w_in[:])

            # ---------------- pass A: sqrt-family ----------------
            stores = []
            last_passA_act = None
            for q0, ch in chunks:
                tcs = ch * T
                rt = wa.tile([128, 3, 1, tcs], F32, tag="rt")
                for qq in range(ch):
                    nc.sync.dma_start(
                        out=rt[:, :, 0, qq * T : (qq + 1) * T],
                        in_=r_in[:, 3 * (q0 + qq) * T : 3 * (q0 + qq + 1) * T].rearrange(
                            "p (c t) -> p c t", c=3
                        ),
                    )
                sq = wa.tile([128, 3, tcs], F32, tag="sq")
                nc.scalar.activation(sq[:], rt[:, :, 0, :], AF.Square)
                x2 = wa.tile([128, 1, tcs], F32, tag="x2")
                nc.vector.tensor_tensor(x2[:], sq[:, 0:1, :], sq[:, 1:2, :], OP.add)
                nc.vector.tensor_tensor(x2[:], x2[:], sq[:, 2:3, :], OP.add)

                st = poolS.tile([128, 1, tcs], F16, tag="s")
                nc.scalar.activation(st[:], x2[:], AF.Sqrt, scale=0.125)
                env = poolE.tile([128, 1, 1, tcs], BF16, tag="env")
                nc.scalar.activation(
                    env[:, :, 0, :], x2[:], AF.Relu, scale=-0.125, bias=1.0
                )
                u = wa.tile([128, 1, tcs], F32, tag="u")
                ua = nc.scalar.activation(u[:], x2[:], AF.Sqrt, bias=64.0 / 289.0)
                last_passA_act = ua
                qr = wa.tile([128, 1, 1, tcs], F32, tag="qr")
                nc.vector.reciprocal(qr[:, :, 0, :], u[:])

                vt = poolV.tile([128, 3, 1, tcs], BF16, tag="v")
                nc.vector.tensor_tensor(
                    vt[:], rt[:], qr[:].to_broadcast([128, 3, 1, tcs]), OP.mult
                )
                stores.append((vt, st, env))

            # ---------------- pass B: trig + matmuls ----------------
            for ci, (q0, ch) in enumerate(chunks):
                tcs = ch * T
                gpc = ch * 8
                vt, st, env = stores[ci]
                own = wc.tile([128, 1, gpc], BF16, tag="own")
                nc.sync.dma_start(
                    out=own[:, 0, :], in_=ow_in[:, q0 * 8 : (q0 + ch) * 8]
                )

                h = wc.tile([128, 7, tcs], F16, tag="h")
                nc.vector.tensor_tensor(
                    h[:], st[:].to_broadcast([128, 7, tcs]), cvt[:, :, :tcs], OP.mult
                )
                rnd = wc.tile([128, 7, tcs], F16, tag="rnd")
                nc.vector.tensor_scalar(
                    rnd[:], h[:], RND_C, RND_C, OP.add, OP.subtract
                )
                z = wc.tile([128, 7, tcs], F16, tag="z")
                nc.vector.tensor_tensor(z[:], h[:], rnd[:], OP.subtract)
                ab = wc.tile([128, 7, tcs], F16, tag="ab")
                ai = nc.scalar.activation(ab[:], z[:], AF.Abs)
                add_dep_helper(
                    ai.ins, last_passA_act.ins, sync=False,
                    reason="keep trig-set ACT ops after all sqrt-set ACT ops",
                )
                radp = wc.tile([128, 1, 7, tcs], BF16, tag="radp")
                nc.scalar.activation(
                    radp[:, 0, :, :],
                    ab[:],
                    AF.Sin,
                    scale=float(-2 * np.pi),
                    bias=float(np.pi / 2),
                )

                phi = wb.tile([128, 10, 8, tcs], BF16, tag="phi")
                nc.vector.tensor_copy(phi[:, 0:1, 0:1, :], env[:])
                nc.vector.tensor_tensor(
                    phi[:, 0:1, 1:8, :], radp[:],
                    env[:].to_broadcast([128, 1, 7, tcs]), OP.mult
                )
                nc.vector.tensor_tensor(
                    phi[:, 1:4],
                    vt[:].to_broadcast([128, 3, 8, tcs]),
                    phi[:, 0:1].to_broadcast([128, 3, 8, tcs]),
                    OP.mult,
                )
                nc.vector.tensor_tensor(
                    phi[:, 4:7],
                    vt[:, 0:1].to_broadcast([128, 3, 8, tcs]),
                    phi[:, 1:4],
                    OP.mult,
                )
                nc.vector.tensor_tensor(
                    phi[:, 7:9],
                    vt[:, 1:2].to_broadcast([128, 2, 8, tcs]),
                    phi[:, 2:4],
                    OP.mult,
                )
                nc.vector.tensor_tensor(
                    phi[:, 9:10],
                    vt[:, 2:3].to_broadcast([128, 1, 8, tcs]),
                    phi[:, 3:4],
                    OP.mult,
                )

                # scatter matrices, one per half-group
                M = wb.tile([128, BLK, gpc], BF16, tag="M")
                nc.vector.tensor_tensor(
                    M[:], own[:].to_broadcast([128, BLK, gpc]), iot[:, :, :gpc],
                    OP.is_equal,
                )

                for pq in range(0, ch, 2):
                    npair = min(2, ch - pq)
                    psq = ps1.tile([128, 2, NF], F32)
                    for par in range(npair):
                        qq = pq + par
                        for g in range(GPB):
                            for tt in range(CAP):
                                for qb in range(4):
                                    grp = qq * 8 + qb * GPB + g
                                    t = qq * T + qb * TPB + g * CAP + tt
                                    nc.tensor.matmul(
                                        psq[qb * BLK : (qb + 1) * BLK, par, :],
                                        M[:, :, grp],
                                        phi[:, :, :, t],
                                        start=(g == 0 and tt == 0),
                                        stop=(g == GPB - 1 and tt == CAP - 1),
                                        tile_position=(0, qb * BLK),
                                    )
                    aq = ob.tile([128, 2, NF], BF16, tag="aq")
                    nc.scalar.copy(aq[:, :npair, :], psq[:, :npair, :])

                    for par in range(npair):
                        qq = pq + par
                        pst = psT.tile([NF, 128], BF16)
                        nc.tensor.transpose(pst[:], aq[:, par, :], idm[:])
                        a2 = ob.tile([NF, 128], BF16, tag="a2")
                        nc.scalar.copy(a2[:], pst[:])

                        po = ps2.tile([128, OUTW], F32)
                        nc.tensor.matmul(po[:], a2[:], wsb[:], start=True, stop=True)
                        osb = ob.tile([128, OUTW], BF16, tag="osb")
                        nc.scalar.copy(osb[:], po[:])
                        nc.sync.dma_start(
                            out=out_p[(q0 + qq) * 128 : (q0 + qq + 1) * 128, :],
                            in_=osb[:],
                        )

    nc.compile()
    return nc


def _get_compiled(nquads):
    if nquads not in _compiled_cache:
        _compiled_cache[nquads] = _build(nquads)
    return _compiled_cache[nquads]


def _pack_nodes(counts, slack=1.015):
    """Pack nodes into blocks: node n takes U=ceil(d/4) capacity-4 units;
    each block holds <=32 nodes and <=256 units. Serpentine deal by U desc
    + fixup; retries with more blocks if packing fails."""
    U = (counts + CAP - 1) // CAP
    total_units = int(U.sum())
    n = counts.shape[0]
    nb_min = max(1, int(np.ceil(total_units / UPB)))
    nb = int(np.ceil(nb_min * slack / 32.0)) * 32
    while nb * BLK < n:
        nb += 32

    order = np.argsort(-U, kind="stable")
    blk_of = np.empty(n, np.int64)
    pos = 0
    ri = 0
    while pos < n:
        take = min(nb, n - pos)
        ids = order[pos : pos + take]
        blocks = np.arange(take) if ri % 2 == 0 else (nb - 1 - np.arange(take))
        blk_of[ids] = blocks
        pos += take
        ri += 1

    bu = np.bincount(blk_of, weights=U.astype(np.float64), minlength=nb).astype(np.int64)
    bn = np.bincount(blk_of, minlength=nb)

    if (bu > UPB).any():
        order_small = order[::-1]
        space = list(np.argsort(bu))
        for b in np.nonzero(bu > UPB)[0]:
            members = order_small[blk_of[order_small] == b]
            k = 0
            while bu[b] > UPB and k < len(members):
                node = members[k]
                k += 1
                un = int(U[node])
                for tb in space:
                    if tb != b and bu[tb] + un <= UPB and bn[tb] < BLK:
                        blk_of[node] = tb
                        bu[b] -= un
                        bu[tb] += un
                        bn[b] -= 1
                        bn[tb] += 1
                        break
                else:
                    return _pack_nodes(counts, slack=slack + 0.01)

    ord2 = np.lexsort((np.arange(n), blk_of))
    blk_sorted = blk_of[ord2]
    bnds = np.append(np.searchsorted(blk_sorted, np.arange(nb)), n)
    sizes = np.diff(bnds)
    Uo = U[ord2]
    cum = np.cumsum(Uo) - Uo
    base_unit = np.empty(n, np.int64)
    local_id = np.empty(n, np.int64)
    base_unit[ord2] = cum - np.repeat(cum[bnds[:-1]], sizes)
    local_id[ord2] = np.arange(n) - np.repeat(bnds[:-1], sizes)
    assert base_unit.max() < UPB and local_id.max() < BLK
    return nb, blk_of, base_unit, local_id


def _preprocess(r_ij, src):
    E = src.shape[0]
    src = np.asarray(src).astype(np.int64).ravel()
    r_ij = np.ascontiguousarray(np.asarray(r_ij, dtype=np.float32))

    counts = np.bincount(src, minlength=N_NODES)
    nb, blk_of, base_unit, local_id = _pack_nodes(counts)
    nbc = nb // N_CORES
    nquads = nbc // 4

    order = np.argsort(src, kind="stable")
    src_s = src[order]
    r_s = r_ij[order]
    starts = np.zeros(N_NODES + 1, np.int64)
    starts[1:] = np.cumsum(counts)
    rank = np.arange(E, dtype=np.int64) - starts[src_s]

    blk = blk_of[src_s]
    unit = base_unit[src_s] + rank // CAP
    g = unit // 128
    p = unit % 128
    tt = g * CAP + rank % CAP
    core = blk // nbc
    bl = blk - core * nbc
    quad = bl // 4
    gt = quad * T + (bl % 4) * TPB + tt

    # pad slots get r=(4,0,0): x2=16 -> env=relu(1-2)=0 -> phi==0 exactly
    r_tmp = np.zeros((N_CORES, 128, 3, nquads * T), np.float32)
    r_tmp[:, :, 0, :] = 4.0
    r_tmp[core, p, :, gt] = r_s
    r_dev = np.ascontiguousarray(
        r_tmp.reshape(N_CORES, 128, 3, nquads, T)
        .transpose(0, 1, 3, 2, 4)
        .reshape(N_CORES, 128, 3 * nquads * T)
    )

    # owner table per (partition, half-group)
    ow = np.full((N_CORES, 128, nbc * GPB), 99.0, np.float32)
    U = (counts + CAP - 1) // CAP
    nodes_rep = np.repeat(np.arange(N_NODES), U)
    un_off = np.arange(U.sum()) - np.repeat(np.cumsum(U) - U, U)
    un = base_unit[nodes_rep] + un_off
    ncore = blk_of // nbc
    nbl = blk_of - ncore * nbc
    ow[ncore[nodes_rep], un % 128, nbl[nodes_rep] * GPB + un // 128] = local_id[
        nodes_rep
    ]

    rows = ncore * (nquads * 128) + (nbl // 4) * 128 + (nbl % 4) * BLK + local_id
    return r_dev, ow.astype(_BF), nquads, rows


def _build_w(W_a, W_v, W_d):
    w = np.zeros((NF, OUTW), np.float32)
    w[0:8, 0:128] = W_a
    for t in range(3):
        w[(1 + t) * 8 : (2 + t) * 8, 128 + 64 * t : 128 + 64 * (t + 1)] = W_v
    for qi in range(6):
        w[(4 + qi) * 8 : (5 + qi) * 8, 320 + 32 * qi : 320 + 32 * (qi + 1)] = W_d
    return w.astype(_BF)


def _make_inputs(r_dev, ow_dev, nquads, W_a, W_v, W_d):
    TC = CHUNK * T
    GPC = CHUNK * 8
    cv = np.ascontiguousarray(
        np.broadcast_to((np.arange(1, 8, dtype=np.float16) * 0.5)[None, :, None], (1, 7, TC))
    )
    io = np.ascontiguousarray(
        np.broadcast_to(np.arange(BLK, dtype=np.float32)[None, :, None], (1, BLK, GPC))
    ).astype(_BF)
    idm = np.eye(128, dtype=np.float32).astype(_BF)
    w = _build_w(np.asarray(W_a, np.float32), np.asarray(W_v, np.float32),
                 np.asarray(W_d, np.float32))
    return [
        dict(r=r_dev[i], ow=ow_dev[i], cv=cv, io=io, idm=idm, w=w)
        for i in range(N_CORES)
    ]


def kernel(r_ij, src, W_a, W_v, W_d, n_nodes):
    from concourse.bass_utils import run_bass_kernel_spmd

    r_dev, ow_dev, nquads, rows = _preprocess(r_ij, src)
    nc = _get_compiled(nquads)
    in_maps = _make_inputs(r_dev, ow_dev, nquads, W_a, W_v, W_d)
    res = run_bass_kernel_spmd(nc, in_maps, core_ids=list(range(N_CORES)))
    full = np.concatenate(
        [np.asarray(res.results[i]["out"]) for i in range(N_CORES)], axis=0
    ).astype(np.float32)[rows]

    N = N_NODES
    B_a = np.ascontiguousarray(full[:, :128])
    B_v = np.ascontiguousarray(
        full[:, 128:320].reshape(N, 3, 64).transpose(0, 2, 1)
    )
    B_d6 = full[:, 320:512].reshape(N, 6, 32)
    pmap = np.array([[0, 1, 2], [1, 3, 4], [2, 4, 5]])
    B_d = np.ascontiguousarray(B_d6[:, pmap, :].transpose(0, 3, 1, 2))
    return B_a, B_v, B_d


# revision 27
# speedup vs baseline: 1.0847x; 1.0644x over previous
"""Trainium2 Bass kernel for nn_ACEEmbedAVD (gnn_message_passing).

Strategy:
  Host: nodes are packed into 128-partition x 8-tile "blocks" (1024 edge
  slots). Each node owns ceil(degree/4) capacity-4 slot units; a unit is
  (partition p, half-group g) holding that node's edges in tiles g*4..g*4+3.
  The scatter matrix M[p, local_node] is therefore constant across the 4
  tiles of a half-group -> built once per half-group (8x less work than a
  per-tile onehot). Unused slots carry r=0 dummy edges whose contribution
  is exactly zero (env factor). No collectives: blocks are sharded across
  the 8 cores; host gathers output rows by node.

  Device (per core), t-innermost layouts for DVE 2x modes:
    pass A (sqrt ACT table): x2=|r|^2; s=sqrt(x2/8); env=relu(1-x2/8) bf16;
      v = r/sqrt(x2+64/289) bf16
    pass B (trig ACT table): rad_c = cos(pi*c*s)*env via round-to-nearest
      big-constant range reduction + Abs + Sin; phi (10x8 feat, bf16);
      M = is_equal(owner, iota32) per half-group; per block
      A_blk (32n, 80f) += M^T @ phi_t in PSUM, col-tiled so a quad's 4
      blocks land in one (128,80) PSUM tile; PE-transpose -> (80,128);
      stage2: B = A^T W_blockdiag (80->512) -> bf16 out rows.

  Host post: rows -> B_a (128), B_v (3x64 -> N,64,3),
  B_d (6 sym pairs x32 -> N,32,3,3 mirrored).
"""

import sys

if "/opt/trn_rl_repo" not in sys.path:
    sys.path.insert(0, "/opt/trn_rl_repo")

import numpy as np
import ml_dtypes

N_NODES = 50000
N_CORES = 8
BLK = 32            # output nodes per block (M columns)
CAP = 4             # edges per slot unit (tiles per half-group)
TPB = 8             # tiles per block
GPB = 2             # half-groups per block
UPB = 128 * GPB     # slot units per block
NF = 80             # phi features
OUTW = 512
CHUNK = 6           # quads per elementwise batch
T = 32              # tiles per quad (4 blocks x 8 tiles)

RND_C = 12582912.0   # 1.5 * 2**23: (x + C) - C == round-to-nearest(x)

_BF = ml_dtypes.bfloat16

_compiled_cache = {}


def _chunk_list(nquads):
    out = []
    q = 0
    while q < nquads:
        ch = min(CHUNK, nquads - q)
        out.append((q, ch))
        q += ch
    return out


def _build(nquads):
    from concourse import bacc, tile, mybir
    from concourse.tile_rust import add_dep_helper

    AF = mybir.ActivationFunctionType
    OP = mybir.AluOpType
    F32 = mybir.dt.float32
    BF16 = mybir.dt.bfloat16
    F16 = mybir.dt.float16

    TC = CHUNK * T        # tiles per full chunk
    GPC = CHUNK * 8       # half-groups per full chunk
    NGRP = nquads * 8     # half-groups per core

    nc = bacc.Bacc("TRN2", target_bir_lowering=False, debug=False)

    def rc(value, dtype=F32):
        key = (dtype, value)
        if key not in nc.const_aps.aps:
            t = nc.alloc_sbuf_tensor(f"c-{dtype.name}-{value}", [128, 1], dtype)
            nc.gpsimd.memset(t.ap(), value)
            nc.const_aps.aps[key] = t.ap()

    rc(64.0 / 289.0)
    rc(float(np.pi / 2))
    rc(1e-20)

    r_in = nc.dram_tensor("r", [128, 3 * nquads * T], F32, kind="ExternalInput")
    ow_in = nc.dram_tensor("ow", [128, NGRP], BF16, kind="ExternalInput")
    cv_in = nc.dram_tensor("cv", [1, 7, TC], F16, kind="ExternalInput")
    io_in = nc.dram_tensor("io", [1, BLK, GPC], BF16, kind="ExternalInput")
    id_in = nc.dram_tensor("idm", [128, 128], BF16, kind="ExternalInput")
    w_in = nc.dram_tensor("w", [NF, OUTW], BF16, kind="ExternalInput")
    out_p = nc.dram_tensor("out", [nquads * 128, OUTW], BF16, kind="ExternalOutput")

    chunks = _chunk_list(nquads)

    with tile.TileContext(nc) as tc:
        with (
            tc.tile_pool(name="const", bufs=1) as cpool,
            tc.tile_pool(name="storeV", bufs=len(chunks)) as poolV,
            tc.tile_pool(name="storeS", bufs=len(chunks)) as poolS,
            tc.tile_pool(name="storeE", bufs=len(chunks)) as poolE,
            tc.tile_pool(name="workA", bufs=3) as wa,
            tc.tile_pool(name="workB", bufs=2) as wb,
            tc.tile_pool(name="workC", bufs=3) as wc,
            tc.tile_pool(name="outB", bufs=3) as ob,
            tc.tile_pool(name="psum1", bufs=3, space="PSUM") as ps1,
            tc.tile_pool(name="psumT", bufs=2, space="PSUM") as psT,
            tc.tile_pool(name="psum2", bufs=2, space="PSUM") as ps2,
        ):
            cvt = cpool.tile([128, 7, TC], F16)
            nc.scalar.dma_start(out=cvt[:], in_=cv_in[:].to_broadcast([128, 7, TC]))
            iot = cpool.tile([128, BLK, GPC], BF16)
            nc.scalar.dma_start(out=iot[:], in_=io_in[:].to_broadcast([128, BLK, GPC]))
            idm = cpool.tile([128, 128], BF16)
            nc.scalar.dma_start(out=idm[:], in_=id_in[:])
            wsb = cpool.tile([NF, OUTW], BF16)
            nc.scalar.dma_start(out=wsb[:], in_=w_in[:])

            # ---------------- pass A: sqrt-family ----------------
            stores = []
            last_passA_act = None
            for q0, ch in chunks:
                tcs = ch * T
                rt = wa.tile([128, 3, 1, tcs], F32, tag="rt")
                for qq in range(ch):
                    nc.sync.dma_start(
                        out=rt[:, :, 0, qq * T : (qq + 1) * T],
                        in_=r_in[:, 3 * (q0 + qq) * T : 3 * (q0 + qq + 1) * T].rearrange(
                            "p (c t) -> p c t", c=3
                        ),
                    )
                sq = wa.tile([128, 3, tcs], F32, tag="sq")
                nc.scalar.activation(sq[:], rt[:, :, 0, :], AF.Square)
                x2 = wa.tile([128, 1, tcs], F32, tag="x2")
                nc.vector.tensor_tensor(x2[:], sq[:, 0:1, :], sq[:, 1:2, :], OP.add)
                nc.vector.tensor_tensor(x2[:], x2[:], sq[:, 2:3, :], OP.add)

                ars = wa.tile([128, 1, tcs], F32, tag="ars")
                nc.scalar.activation(
                    ars[:], x2[:], AF.Abs_reciprocal_sqrt, scale=8.0, bias=1e-20
                )
                st = poolS.tile([128, 1, tcs], F16, tag="s")
                nc.vector.tensor_tensor(st[:], x2[:], ars[:], OP.mult)
                env = poolE.tile([128, 1, 1, tcs], BF16, tag="env")
                nc.scalar.activation(
                    env[:, :, 0, :], x2[:], AF.Relu, scale=-0.125, bias=1.0
                )
                qr = wa.tile([128, 1, 1, tcs], F32, tag="qr")
                ua = nc.scalar.activation(
                    qr[:, :, 0, :], x2[:], AF.Abs_reciprocal_sqrt, bias=64.0 / 289.0
                )
                last_passA_act = ua

                vt = poolV.tile([128, 3, 1, tcs], BF16, tag="v")
                nc.vector.tensor_tensor(
                    vt[:], rt[:], qr[:].to_broadcast([128, 3, 1, tcs]), OP.mult
                )
                stores.append((vt, st, env))

            # ---------------- pass B: trig + matmuls ----------------
            for ci, (q0, ch) in enumerate(chunks):
                tcs = ch * T
                gpc = ch * 8
                vt, st, env = stores[ci]
                own = wc.tile([128, 1, gpc], BF16, tag="own")
                nc.sync.dma_start(
                    out=own[:, 0, :], in_=ow_in[:, q0 * 8 : (q0 + ch) * 8]
                )

                h = wc.tile([128, 7, tcs], F16, tag="h")
                nc.vector.tensor_tensor(
                    h[:], st[:].to_broadcast([128, 7, tcs]), cvt[:, :, :tcs], OP.mult
                )
                rnd = wc.tile([128, 7, tcs], F16, tag="rnd")
                nc.vector.tensor_scalar(
                    rnd[:], h[:], RND_C, RND_C, OP.add, OP.subtract
                )
                z = wc.tile([128, 7, tcs], F16, tag="z")
                nc.vector.tensor_tensor(z[:], h[:], rnd[:], OP.subtract)
                ab = wc.tile([128, 7, tcs], F16, tag="ab")
                nc.scalar.activation(ab[:], z[:], AF.Abs)
                radp = wc.tile([128, 1, 7, tcs], BF16, tag="radp")
                si = nc.scalar.activation(
                    radp[:, 0, :, :],
                    ab[:],
                    AF.Sin,
                    scale=float(-2 * np.pi),
                    bias=float(np.pi / 2),
                )
                add_dep_helper(
                    si.ins, last_passA_act.ins, sync=False,
                    reason="keep trig-set ACT ops after all rsqrt-set ACT ops",
                )

                phi = wb.tile([128, 10, 8, tcs], BF16, tag="phi")
                nc.vector.tensor_copy(phi[:, 0:1, 0:1, :], env[:])
                nc.vector.tensor_tensor(
                    phi[:, 0:1, 1:8, :], radp[:],
                    env[:].to_broadcast([128, 1, 7, tcs]), OP.mult
                )
                nc.vector.tensor_tensor(
                    phi[:, 1:4],
                    vt[:].to_broadcast([128, 3, 8, tcs]),
                    phi[:, 0:1].to_broadcast([128, 3, 8, tcs]),
                    OP.mult,
                )
                nc.vector.tensor_tensor(
                    phi[:, 4:7],
                    vt[:, 0:1].to_broadcast([128, 3, 8, tcs]),
                    phi[:, 1:4],
                    OP.mult,
                )
                nc.vector.tensor_tensor(
                    phi[:, 7:9],
                    vt[:, 1:2].to_broadcast([128, 2, 8, tcs]),
                    phi[:, 2:4],
                    OP.mult,
                )
                nc.vector.tensor_tensor(
                    phi[:, 9:10],
                    vt[:, 2:3].to_broadcast([128, 1, 8, tcs]),
                    phi[:, 3:4],
                    OP.mult,
                )

                # scatter matrices, one per half-group
                M = wb.tile([128, BLK, gpc], BF16, tag="M")
                nc.vector.tensor_tensor(
                    M[:], own[:].to_broadcast([128, BLK, gpc]), iot[:, :, :gpc],
                    OP.is_equal,
                )

                for pq in range(0, ch, 4):
                    npair = min(4, ch - pq)
                    psq = ps1.tile([128, 4, NF], F32)
                    for par in range(npair):
                        qq = pq + par
                        for g in range(GPB):
                            for tt in range(CAP):
                                for qb in range(4):
                                    grp = qq * 8 + qb * GPB + g
                                    t = qq * T + qb * TPB + g * CAP + tt
                                    nc.tensor.matmul(
                                        psq[qb * BLK : (qb + 1) * BLK, par, :],
                                        M[:, :, grp],
                                        phi[:, :, :, t],
                                        start=(g == 0 and tt == 0),
                                        stop=(g == GPB - 1 and tt == CAP - 1),
                                        tile_position=(0, qb * BLK),
                                    )
                    aq = ob.tile([128, 4, NF], BF16, tag="aq")
                    nc.scalar.copy(aq[:, :npair, :], psq[:, :npair, :])

                    for par in range(npair):
                        qq = pq + par
                        pst = psT.tile([NF, 128], BF16)
                        nc.tensor.transpose(pst[:], aq[:, par, :], idm[:])
                        a2 = ob.tile([NF, 128], BF16, tag="a2")
                        nc.scalar.copy(a2[:], pst[:])

                        po = ps2.tile([128, OUTW], F32)
                        nc.tensor.matmul(po[:], a2[:], wsb[:], start=True, stop=True)
                        osb = ob.tile([128, OUTW], BF16, tag="osb")
                        nc.scalar.copy(osb[:], po[:])
                        nc.sync.dma_start(
                            out=out_p[(q0 + qq) * 128 : (q0 + qq + 1) * 128, :],
                            in_=osb[:],
                        )

    nc.compile()
    return nc


def _get_compiled(nquads):
    if nquads not in _compiled_cache:
        _compiled_cache[nquads] = _build(nquads)
    return _compiled_cache[nquads]


def _pack_nodes(counts, slack=1.015):
    """Pack nodes into blocks: node n takes U=ceil(d/4) capacity-4 units;
    each block holds <=32 nodes and <=256 units. Serpentine deal by U desc
    + fixup; retries with more blocks if packing fails."""
    U = (counts + CAP - 1) // CAP
    total_units = int(U.sum())
    n = counts.shape[0]
    nb_min = max(1, int(np.ceil(total_units / UPB)))
    nb = int(np.ceil(nb_min * slack / 32.0)) * 32
    while nb * BLK < n:
        nb += 32

    order = np.argsort(-U, kind="stable")
    blk_of = np.empty(n, np.int64)
    pos = 0
    ri = 0
    while pos < n:
        take = min(nb, n - pos)
        ids = order[pos : pos + take]
        blocks = np.arange(take) if ri % 2 == 0 else (nb - 1 - np.arange(take))
        blk_of[ids] = blocks
        pos += take
        ri += 1

    bu = np.bincount(blk_of, weights=U.astype(np.float64), minlength=nb).astype(np.int64)
    bn = np.bincount(blk_of, minlength=nb)

    if (bu > UPB).any():
        order_small = order[::-1]
        space = list(np.argsort(bu))
        for b in np.nonzero(bu > UPB)[0]:
            members = order_small[blk_of[order_small] == b]
            k = 0
            while bu[b] > UPB and k < len(members):
                node = members[k]
                k += 1
                un = int(U[node])
                for tb in space:
                    if tb != b and bu[tb] + un <= UPB and bn[tb] < BLK:
                        blk_of[node] = tb
                        bu[b] -= un
                        bu[tb] += un
                        bn[b] -= 1
                        bn[tb] += 1
                        break
                else:
                    return _pack_nodes(counts, slack=slack + 0.01)

    ord2 = np.lexsort((np.arange(n), blk_of))
    blk_sorted = blk_of[ord2]
    bnds = np.append(np.searchsorted(blk_sorted, np.arange(nb)), n)
    sizes = np.diff(bnds)
    Uo = U[ord2]
    cum = np.cumsum(Uo) - Uo
    base_unit = np.empty(n, np.int64)
    local_id = np.empty(n, np.int64)
    base_unit[ord2] = cum - np.repeat(cum[bnds[:-1]], sizes)
    local_id[ord2] = np.arange(n) - np.repeat(bnds[:-1], sizes)
    assert base_unit.max() < UPB and local_id.max() < BLK
    return nb, blk_of, base_unit, local_id


def _preprocess(r_ij, src):
    E = src.shape[0]
    src = np.asarray(src).astype(np.int64).ravel()
    r_ij = np.ascontiguousarray(np.asarray(r_ij, dtype=np.float32))

    counts = np.bincount(src, minlength=N_NODES)
    nb, blk_of, base_unit, local_id = _pack_nodes(counts)
    nbc = nb // N_CORES
    nquads = nbc // 4

    order = np.argsort(src, kind="stable")
    src_s = src[order]
    r_s = r_ij[order]
    starts = np.zeros(N_NODES + 1, np.int64)
    starts[1:] = np.cumsum(counts)
    rank = np.arange(E, dtype=np.int64) - starts[src_s]

    blk = blk_of[src_s]
    unit = base_unit[src_s] + rank // CAP
    g = unit // 128
    p = unit % 128
    tt = g * CAP + rank % CAP
    core = blk // nbc
    bl = blk - core * nbc
    quad = bl // 4
    gt = quad * T + (bl % 4) * TPB + tt

    # pad slots get r=(4,0,0): x2=16 -> env=relu(1-2)=0 -> phi==0 exactly
    r_tmp = np.zeros((N_CORES, 128, 3, nquads * T), np.float32)
    r_tmp[:, :, 0, :] = 4.0
    r_tmp[core, p, :, gt] = r_s
    r_dev = np.ascontiguousarray(
        r_tmp.reshape(N_CORES, 128, 3, nquads, T)
        .transpose(0, 1, 3, 2, 4)
        .reshape(N_CORES, 128, 3 * nquads * T)
    )

    # owner table per (partition, half-group)
    ow = np.full((N_CORES, 128, nbc * GPB), 99.0, np.float32)
    U = (counts + CAP - 1) // CAP
    nodes_rep = np.repeat(np.arange(N_NODES), U)
    un_off = np.arange(U.sum()) - np.repeat(np.cumsum(U) - U, U)
    un = base_unit[nodes_rep] + un_off
    ncore = blk_of // nbc
    nbl = blk_of - ncore * nbc
    ow[ncore[nodes_rep], un % 128, nbl[nodes_rep] * GPB + un // 128] = local_id[
        nodes_rep
    ]

    rows = ncore * (nquads * 128) + (nbl // 4) * 128 + (nbl % 4) * BLK + local_id
    return r_dev, ow.astype(_BF), nquads, rows


def _build_w(W_a, W_v, W_d):
    w = np.zeros((NF, OUTW), np.float32)
    w[0:8, 0:128] = W_a
    for t in range(3):
        w[(1 + t) * 8 : (2 + t) * 8, 128 + 64 * t : 128 + 64 * (t + 1)] = W_v
    for qi in range(6):
        w[(4 + qi) * 8 : (5 + qi) * 8, 320 + 32 * qi : 320 + 32 * (qi + 1)] = W_d
    return w.astype(_BF)


def _make_inputs(r_dev, ow_dev, nquads, W_a, W_v, W_d):
    TC = CHUNK * T
    GPC = CHUNK * 8
    cv = np.ascontiguousarray(
        np.broadcast_to((np.arange(1, 8, dtype=np.float16) * 0.5)[None, :, None], (1, 7, TC))
    )
    io = np.ascontiguousarray(
        np.broadcast_to(np.arange(BLK, dtype=np.float32)[None, :, None], (1, BLK, GPC))
    ).astype(_BF)
    idm = np.eye(128, dtype=np.float32).astype(_BF)
    w = _build_w(np.asarray(W_a, np.float32), np.asarray(W_v, np.float32),
                 np.asarray(W_d, np.float32))
    return [
        dict(r=r_dev[i], ow=ow_dev[i], cv=cv, io=io, idm=idm, w=w)
        for i in range(N_CORES)
    ]


def kernel(r_ij, src, W_a, W_v, W_d, n_nodes):
    from concourse.bass_utils import run_bass_kernel_spmd

    r_dev, ow_dev, nquads, rows = _preprocess(r_ij, src)
    nc = _get_compiled(nquads)
    in_maps = _make_inputs(r_dev, ow_dev, nquads, W_a, W_v, W_d)
    res = run_bass_kernel_spmd(nc, in_maps, core_ids=list(range(N_CORES)))
    full = np.concatenate(
        [np.asarray(res.results[i]["out"]) for i in range(N_CORES)], axis=0
    ).astype(np.float32)[rows]

    N = N_NODES
    B_a = np.ascontiguousarray(full[:, :128])
    B_v = np.ascontiguousarray(
        full[:, 128:320].reshape(N, 3, 64).transpose(0, 2, 1)
    )
    B_d6 = full[:, 320:512].reshape(N, 6, 32)
    pmap = np.array([[0, 1, 2], [1, 3, 4], [2, 4, 5]])
    B_d = np.ascontiguousarray(B_d6[:, pmap, :].transpose(0, 3, 1, 2))
    return B_a, B_v, B_d
